# revision 6
# baseline (speedup 1.0000x reference)
"""Multi-head attention (B=4, N=2048, C=768, H=12, Dh=64) on 8 TRN2 NeuronCores.

Sharding: tensor-parallel on heads. 2 cores per batch; each core owns 6 of the
12 heads over the FULL 2048-token sequence, so no K/V projection work is
duplicated (a query-split layout computes each batch's K/V twice). Each core
emits a partial projection output [2048, 768] (its heads' slice of the
contraction); the host sums the two partials per batch and adds the bias as
the unshard step (the spec's "all-reduce after proj" done host-side).

Per-core inputs (partition dim first):
  xT     [768, 2048]  bf16  x[b].T
  wqkvT  [768, 1152]  bf16  [q | k | v] columns for this core's 6 heads
  wprojT [384, 768]   bf16  proj_w.T rows for this core's 6 heads
  out    [2048, 768]  f32   partial projection output

Pipeline (identical inner structure to the measured-good query-split kernel;
keeping the PE duty profile under the power limiter's threshold matters more
than back-to-back PE saturation — an over-pipelined variant measured 477 us
because the firmware clamped the PE to K=4/8 for 37% of the kernel):
  - qkv projection in bf16 (psum f32), all upfront; q/k stored transposed
    [d, n] packed two heads per 128-partition tile; v stored [token, d]
    augmented with a ones column per head ([64 v | 1] x 6 heads).
  - scores: S^T chunks [128 kv-rows, 512 q] = kT-slice.T @ qT-slice (K=64);
    exp on ScalarE over [128, <=1536] PSUM spans with the 1/sqrt(Dh) scale
    folded into the ACTIVATE affine (no max-subtraction; |S|<=~10 here).
  - PV: one matmul per (kv-chunk, head) with augmented V stationary [128, 65]
    -> attn-out.T rows 0..63 + softmax denominator at row 64 of the same PSUM
    accumulation group.
  - normalize: denominator row -> reciprocal_approx_fast -> gpsimd partition
    broadcast -> vector multiply -> bf16 attnT tiles (proj-ready layout).
  - projection per 512-row query block (overlaps the next block's attention).
"""

import sys

if "/opt/trn_rl_repo" not in sys.path:
    sys.path.insert(0, "/opt/trn_rl_repo")

import numpy as np
import ml_dtypes

B, N, C = 4, 2048, 768
H, Dh = 12, 64
HC = H // 2            # heads per core
HPC = HC // 2          # head-pairs per core (3)
CW = HC * Dh           # 384 output-d columns per core
SCALE = Dh ** -0.5
CCH = C // 128         # 6 contraction chunks
NCORES = 8
JG = [(2 * i, 2) for i in range(8)]  # kv j-block groups (2 x 128 rows)

_NC_CACHE = {}


def _build():
    import concourse.bass as bass
    import concourse.tile as tile
    import concourse.mybir as mybir
    from concourse import bacc

    f32 = mybir.dt.float32
    bf16 = mybir.dt.bfloat16
    Exp = mybir.ActivationFunctionType.Exp

    nc = bacc.Bacc(
        "TRN2",
        target_bir_lowering=False,
        debug=False,
        enable_asserts=False,
        num_devices=NCORES,
    )

    xT = nc.dram_tensor("xT", [C, N], bf16, kind="ExternalInput").ap()
    wqkvT = nc.dram_tensor("wqkvT", [C, 3 * CW], bf16, kind="ExternalInput").ap()
    wprojT = nc.dram_tensor("wprojT", [CW, C], bf16, kind="ExternalInput").ap()
    out = nc.dram_tensor("out", [N, C], f32, kind="ExternalOutput").ap()

    with tile.TileContext(nc) as tc:
        from contextlib import ExitStack

        with ExitStack() as ctx:
            singles = ctx.enter_context(tc.tile_pool(name="singles", bufs=1))
            psum = ctx.enter_context(tc.tile_pool(name="psum", bufs=1, space="PSUM"))

            # ---- load phase-A inputs (released after qkv) ---------------
            load = tc.alloc_tile_pool(name="load", bufs=1)
            xt = [load.tile([128, N], bf16, tag=f"xt{i}", name=f"xt{i}")
                  for i in range(CCH)]
            wq = [load.tile([128, 3 * CW], bf16, tag=f"wq{i}", name=f"wq{i}")
                  for i in range(CCH)]
            for i in range(CCH):
                nc.gpsimd.dma_start(out=wq[i][:, 0:128],
                                    in_=wqkvT[i * 128:(i + 1) * 128, 0:128])
            for i in range(CCH):
                nc.gpsimd.dma_start(out=wq[i][:, 128:],
                                    in_=wqkvT[i * 128:(i + 1) * 128, 128:])
            for nch in range(N // 512):
                for i in range(CCH):
                    nc.sync.dma_start(
                        out=xt[i][:, nch * 512:(nch + 1) * 512],
                        in_=xT[i * 128:(i + 1) * 128, nch * 512:(nch + 1) * 512])
            wp = []
            for i in range(HPC):
                t = singles.tile([128, C], bf16, tag=f"wp{i}", name=f"wp{i}")
                nc.gpsimd.dma_start(out=t, in_=wprojT[i * 128:(i + 1) * 128, :])
                wp.append(t)

            # ---- qkv projections, results stored bf16 -------------------
            qt = [singles.tile([128, N], bf16, tag=f"qt{i}", name=f"qt{i}")
                  for i in range(HPC)]
            kt = [singles.tile([128, N], bf16, tag=f"kt{i}", name=f"kt{i}")
                  for i in range(HPC)]
            # v_aug: per 128-token tile, 6 heads x (64 v-cols + ones col)
            vt = [singles.tile([128, HC * 65], bf16, tag=f"vt{i}", name=f"vt{i}")
                  for i in range(N // 128)]

            # qT[d, n] and kT[d, n], two heads per 128-partition tile
            for dt in range(HPC):
                for nch in range(N // 512):
                    ps = psum.tile([128, 512], f32, tag="st", bufs=2, name="ps_q")
                    for cc in range(CCH):
                        nc.tensor.matmul(
                            ps,
                            lhsT=wq[cc][:, dt * 128:(dt + 1) * 128],
                            rhs=xt[cc][:, nch * 512:(nch + 1) * 512],
                            start=(cc == 0), stop=(cc == CCH - 1),
                        )
                    nc.vector.tensor_copy(qt[dt][:, nch * 512:(nch + 1) * 512], ps)
                for nch in range(N // 512):
                    ps = psum.tile([128, 512], f32, tag="st", bufs=2, name="ps_k")
                    for cc in range(CCH):
                        nc.tensor.matmul(
                            ps,
                            lhsT=wq[cc][:, CW + dt * 128:CW + (dt + 1) * 128],
                            rhs=xt[cc][:, nch * 512:(nch + 1) * 512],
                            start=(cc == 0), stop=(cc == CCH - 1),
                        )
                    nc.vector.tensor_copy(kt[dt][:, nch * 512:(nch + 1) * 512], ps)

            # v in [token, d] layout: v[n, d] = sum_c xT[c, n] * wv[c, d]
            for nt in range(N // 128):
                vaug = vt[nt].rearrange("p (h e) -> p h e", e=65)
                nc.vector.memset(vaug[:, :, 64:65], 1.0)
                ps = psum.tile([128, 512], f32, tag="st", bufs=2, name="ps_v")
                for cc in range(CCH):
                    nc.tensor.matmul(
                        ps[:, :CW],
                        lhsT=xt[cc][:, nt * 128:(nt + 1) * 128],
                        rhs=wq[cc][:, 2 * CW:3 * CW],
                        start=(cc == 0), stop=(cc == CCH - 1),
                    )
                nc.vector.tensor_copy(
                    vaug[:, :, 0:64],
                    ps[:, :CW].rearrange("p (h e) -> p h e", e=64),
                )

            load.release()

            # ---- attention ----------------------------------------------
            work = ctx.enter_context(tc.tile_pool(name="work", bufs=4))
            att = [singles.tile([128, N], bf16, tag=f"att{i}", name=f"att{i}")
                   for i in range(HPC)]

            def proj_block(ic):
                """Projection for one 128-row block (partial: this core's d)."""
                pj = psum.tile([128, C], f32, tag="st", bufs=2, name="pj")
                for (d0, dw) in ((0, 512), (512, 256)):
                    for dt in range(HPC):
                        nc.tensor.matmul(
                            pj[:, d0:d0 + dw],
                            lhsT=att[dt][:, ic * 128:(ic + 1) * 128],
                            rhs=wp[dt][:, d0:d0 + dw],
                            start=(dt == 0), stop=(dt == HPC - 1),
                        )
                osb = work.tile([128, C], f32, tag="osb", bufs=3, name="osb")
                nc.vector.tensor_copy(osb, pj)
                nc.sync.dma_start(out=out[ic * 128:(ic + 1) * 128, :], in_=osb)

            proj_ready = []          # row blocks whose query block is done
            for qb in range(N // 512):           # 512-wide query block
                for hp in range(HPC):            # head pair
                    pv = []
                    for h2 in range(2):
                        pv.append(psum.tile([128, 512], f32, tag="pv",
                                            bufs=4, name=f"pv{h2}"))
                    for (j0, jn) in JG:          # j-groups of 2x128 kv rows
                        for h2 in range(2):
                            hb = h2 * 64
                            st = psum.tile([128, 1024], f32, tag="st", bufs=2,
                                           name="st")
                            for cx in range(jn):
                                j = j0 + cx
                                nc.tensor.matmul(
                                    st[:, cx * 512:(cx + 1) * 512],
                                    lhsT=kt[hp][hb:hb + 64, j * 128:(j + 1) * 128],
                                    rhs=qt[hp][hb:hb + 64, qb * 512:(qb + 1) * 512],
                                    start=True, stop=True,
                                )
                            et = work.tile([128, 1024], bf16, tag="et", bufs=8,
                                           name="et")
                            nc.scalar.activation(et[:, :jn * 512],
                                                 st[:, :jn * 512],
                                                 Exp, scale=SCALE)
                            for cx in range(jn):
                                j = j0 + cx
                                h = hp * 2 + h2
                                nc.tensor.matmul(
                                    pv[h2][0:65, :],
                                    lhsT=vt[j][:, h * 65:(h + 1) * 65],
                                    rhs=et[:, cx * 512:(cx + 1) * 512],
                                    start=(j == 0), stop=(j == N // 128 - 1),
                                )
                    for h2 in range(2):
                        srow = work.tile([1, 512], f32, tag="srow",
                                         bufs=4, name="srow")
                        nc.vector.tensor_copy(srow, pv[h2][64:65, :])
                        sinv = work.tile([1, 512], f32, tag="sinv",
                                         bufs=4, name="sinv")
                        nc.vector.reciprocal_approx_fast(sinv, srow)
                        bc = work.tile([64, 512], f32, tag="bc",
                                       bufs=4, name="bc")
                        nc.gpsimd.partition_broadcast(bc, sinv)
                        nc.vector.tensor_mul(
                            att[hp][h2 * 64:h2 * 64 + 64, qb * 512:(qb + 1) * 512],
                            pv[h2][0:64, :],
                            bc,
                        )
                    # interleave ready projection blocks to keep ScalarE from
                    # idling at query-block boundaries (2/unit in the last qb)
                    for _ in range(2 if qb == N // 512 - 1 else 1):
                        if proj_ready:
                            proj_block(proj_ready.pop(0))
                proj_ready.extend(range(qb * 4, qb * 4 + 4))
            for ic in proj_ready:
                proj_block(ic)

    nc.compile()
    return nc


def _get_nc():
    if "nc" not in _NC_CACHE:
        _NC_CACHE["nc"] = _build()
    return _NC_CACHE["nc"]


def _ensure_ntff_hook():
    """The agent image's ``antenv`` lacks ``axon_hooks``; synthesize it so
    ``run_bass_kernel_spmd(trace=True)`` can capture NTFF profiles."""
    import types
    try:
        from antenv.axon_hooks import get_axon_ntff_profile_hook  # noqa: F401
        return
    except ImportError:
        pass
    import antenv
    from trn_agent_boot.trn_boot import _ntff_profile_via_ctypes
    hook = _ntff_profile_via_ctypes("/opt/axon/libaxon_pjrt.so")
    mod = types.ModuleType("antenv.axon_hooks")
    mod._hook = hook
    mod.get_axon_ntff_profile_hook = lambda: mod._hook

    def _set(h):
        mod._hook = h

    mod.set_axon_ntff_profile_hook = _set
    sys.modules["antenv.axon_hooks"] = mod
    antenv.axon_hooks = mod


def kernel(trace=False, **inputs):
    x = np.asarray(inputs["x"], np.float32)
    qkv_w = np.asarray(inputs["qkv_w"], np.float32)
    proj_w = np.asarray(inputs["proj_w"], np.float32)
    proj_b = np.asarray(inputs["proj_b"], np.float32)

    nc = _get_nc()

    xTb = np.ascontiguousarray(x.transpose(0, 2, 1)).astype(ml_dtypes.bfloat16)
    wqkvT = np.ascontiguousarray(qkv_w.T).astype(ml_dtypes.bfloat16)
    wprojT = np.ascontiguousarray(proj_w.T).astype(ml_dtypes.bfloat16)

    in_maps = []
    for c in range(NCORES):
        b, hg = divmod(c, 2)
        cs = hg * CW
        wslice = np.concatenate(
            [wqkvT[:, s + cs:s + cs + CW] for s in (0, C, 2 * C)], axis=1)
        in_maps.append({
            "xT": xTb[b],
            "wqkvT": np.ascontiguousarray(wslice),
            "wprojT": np.ascontiguousarray(wprojT[cs:cs + CW, :]),
        })

    from concourse import bass_utils
    if trace:
        _ensure_ntff_hook()
        bass_utils.upload_artifacts = lambda tmpdir: tmpdir
    res = bass_utils.run_bass_kernel_spmd(
        nc, in_maps, core_ids=list(range(NCORES)), trace=trace,
    )

    out = np.empty((B, N, C), np.float32)
    for b in range(B):
        out[b] = res.results[2 * b]["out"] + res.results[2 * b + 1]["out"]
    out += proj_b

    if trace:
        return out, res
    return out


# revision 7
# speedup vs baseline: 1.1042x; 1.1042x over previous
"""Multi-head attention (B=4, N=2048, C=768, H=12, Dh=64) on 8 TRN2 NeuronCores.

Sharding: tensor-parallel on heads. 2 cores per batch; each core owns 6 of the
12 heads over the FULL 2048-token sequence, so no K/V projection work is
duplicated. Each core emits a partial projection output [2048, 768]; the host
sums the two partials per batch and adds the bias as the unshard step.

Schedule notes (power limiter!): the firmware clamps the PE to K=4/8 when
sustained PE duty is too high — an over-pipelined variant (qkv interleaved
into attention everywhere) measured 477 us vs 373 for the same cycle count,
and a proj-interleaved variant 412. The schedule here keeps the attention
phase ACT(exp)-paced at ~80% PE duty:
  - qkv is issued in blocks: a minimal prefix (k+q for head-pair 0, all of v)
    unblocks attention unit (qb0, hp0); the remaining q/k groups run as two
    blocks between the first three units, where ScalarE still has queued exp
    work, so the first exp starts at ~30 us instead of ~70.
  - within a unit the two heads run as separate score/exp/PV streams, so each
    head's softmax-normalize (DVE+Pool chain) overlaps the other head's
    stream instead of stalling the next unit's PV accumulation.
  - projection for query-block qb runs inside unit (qb+1, hp0), split at
    points where ScalarE has exp backlog; the last block's projection tails.
Inner structure (measured-good): scores S^T [128 kv, 512 q] (K=64), exp over
[128, <=1536] PSUM spans with 1/sqrt(Dh) folded in, PV with ones-augmented V
stationary [128, 65] (softmax denominator = PSUM row 64), normalize via
reciprocal + gpsimd partition-broadcast. PSUM: st 2x3 banks + pv 2x1 = 8.
"""

import sys

if "/opt/trn_rl_repo" not in sys.path:
    sys.path.insert(0, "/opt/trn_rl_repo")

import numpy as np
import ml_dtypes

B, N, C = 4, 2048, 768
H, Dh = 12, 64
HC = H // 2            # heads per core
HPC = HC // 2          # head-pairs per core (3)
CW = HC * Dh           # 384 output-d columns per core
SCALE = Dh ** -0.5
CCH = C // 128         # 6 contraction chunks
NCORES = 8
JG = [(0, 3), (3, 3), (6, 3), (9, 3), (12, 2), (14, 2)]  # kv j-block groups

_NC_CACHE = {}


def _build():
    import concourse.bass as bass
    import concourse.tile as tile
    import concourse.mybir as mybir
    from concourse import bacc

    f32 = mybir.dt.float32
    bf16 = mybir.dt.bfloat16
    Exp = mybir.ActivationFunctionType.Exp

    nc = bacc.Bacc(
        "TRN2",
        target_bir_lowering=False,
        debug=False,
        enable_asserts=False,
        num_devices=NCORES,
    )

    xT = nc.dram_tensor("xT", [C, N], bf16, kind="ExternalInput").ap()
    wqkvT = nc.dram_tensor("wqkvT", [C, 3 * CW], bf16, kind="ExternalInput").ap()
    wprojT = nc.dram_tensor("wprojT", [CW, C], bf16, kind="ExternalInput").ap()
    out = nc.dram_tensor("out", [N, C], f32, kind="ExternalOutput").ap()

    with tile.TileContext(nc) as tc:
        from contextlib import ExitStack

        with ExitStack() as ctx:
            singles = ctx.enter_context(tc.tile_pool(name="singles", bufs=1))
            psum = ctx.enter_context(tc.tile_pool(name="psum", bufs=1, space="PSUM"))

            # ---- input tiles (kept resident; ~11 MB total SBUF) ---------
            xt = [singles.tile([128, N], bf16, tag=f"xt{i}", name=f"xt{i}")
                  for i in range(CCH)]
            wq = [singles.tile([128, 3 * CW], bf16, tag=f"wq{i}", name=f"wq{i}")
                  for i in range(CCH)]
            # k(hp0) cols first so the first projection group can start early
            for i in range(CCH):
                nc.gpsimd.dma_start(out=wq[i][:, CW:CW + 128],
                                    in_=wqkvT[i * 128:(i + 1) * 128, CW:CW + 128])
            for i in range(CCH):
                nc.gpsimd.dma_start(out=wq[i][:, 0:CW],
                                    in_=wqkvT[i * 128:(i + 1) * 128, 0:CW])
            for i in range(CCH):
                nc.gpsimd.dma_start(out=wq[i][:, CW + 128:],
                                    in_=wqkvT[i * 128:(i + 1) * 128, CW + 128:])
            for nch in range(N // 512):
                for i in range(CCH):
                    nc.sync.dma_start(
                        out=xt[i][:, nch * 512:(nch + 1) * 512],
                        in_=xT[i * 128:(i + 1) * 128, nch * 512:(nch + 1) * 512])
            wp = []
            for i in range(HPC):
                t = singles.tile([128, C], bf16, tag=f"wp{i}", name=f"wp{i}")
                nc.gpsimd.dma_start(out=t, in_=wprojT[i * 128:(i + 1) * 128, :])
                wp.append(t)

            # ---- result tiles ------------------------------------------
            qt = [singles.tile([128, N], bf16, tag=f"qt{i}", name=f"qt{i}")
                  for i in range(HPC)]
            kt = [singles.tile([128, N], bf16, tag=f"kt{i}", name=f"kt{i}")
                  for i in range(HPC)]
            vt = [singles.tile([128, HC * 65], bf16, tag=f"vt{i}", name=f"vt{i}")
                  for i in range(N // 128)]
            att = [singles.tile([128, N], bf16, tag=f"att{i}", name=f"att{i}")
                   for i in range(HPC)]

            def qk_group(dst, col0, nch):
                """One psum group of a q/k projection: 512 tokens x 128 d."""
                ps = psum.tile([128, 512], f32, tag="st", bufs=2, name="ps_qk")
                for cc in range(CCH):
                    nc.tensor.matmul(
                        ps,
                        lhsT=wq[cc][:, col0:col0 + 128],
                        rhs=xt[cc][:, nch * 512:(nch + 1) * 512],
                        start=(cc == 0), stop=(cc == CCH - 1),
                    )
                nc.vector.tensor_copy(dst[:, nch * 512:(nch + 1) * 512], ps)

            def v_group(nt):
                """v for one 128-token tile, all 6 heads, augmented layout."""
                vaug = vt[nt].rearrange("p (h e) -> p h e", e=65)
                nc.vector.memset(vaug[:, :, 64:65], 1.0)
                ps = psum.tile([128, 512], f32, tag="st", bufs=2, name="ps_v")
                for cc in range(CCH):
                    nc.tensor.matmul(
                        ps[:, :CW],
                        lhsT=xt[cc][:, nt * 128:(nt + 1) * 128],
                        rhs=wq[cc][:, 2 * CW:3 * CW],
                        start=(cc == 0), stop=(cc == CCH - 1),
                    )
                nc.vector.tensor_copy(
                    vaug[:, :, 0:64],
                    ps[:, :CW].rearrange("p (h e) -> p h e", e=64),
                )

            work = ctx.enter_context(tc.tile_pool(name="work", bufs=4))

            def proj_block(ic):
                """Projection for one 128-row block (partial: this core's d)."""
                pj = psum.tile([128, C], f32, tag="st", bufs=2, name="pj")
                for (d0, dw) in ((0, 512), (512, 256)):
                    for dt in range(HPC):
                        nc.tensor.matmul(
                            pj[:, d0:d0 + dw],
                            lhsT=att[dt][:, ic * 128:(ic + 1) * 128],
                            rhs=wp[dt][:, d0:d0 + dw],
                            start=(dt == 0), stop=(dt == HPC - 1),
                        )
                osb = work.tile([128, C], f32, tag="osb", bufs=3, name="osb")
                nc.vector.tensor_copy(osb, pj)
                nc.sync.dma_start(out=out[ic * 128:(ic + 1) * 128, :], in_=osb)

            def attn_unit(qb, hp, proj_blocks=()):
                """Scores+exp+PV+normalize, one 512-q block x head pair.

                The two heads run as separate streams so each normalize
                overlaps the other stream. proj_blocks (row indices) are
                drained mid-stream where ScalarE has exp backlog.
                """
                pv = [psum.tile([128, 512], f32, tag="pv", bufs=2,
                                name=f"pv{h2}") for h2 in range(2)]
                pb = list(proj_blocks)
                for h2 in range(2):
                    hb = h2 * 64
                    for gi, (j0, jn) in enumerate(JG):
                        st = psum.tile([128, 1536], f32, tag="st", bufs=2,
                                       name="st")
                        for cx in range(jn):
                            j = j0 + cx
                            nc.tensor.matmul(
                                st[:, cx * 512:(cx + 1) * 512],
                                lhsT=kt[hp][hb:hb + 64, j * 128:(j + 1) * 128],
                                rhs=qt[hp][hb:hb + 64, qb * 512:(qb + 1) * 512],
                                start=True, stop=True,
                            )
                        et = work.tile([128, 1536], bf16, tag="et", bufs=6,
                                       name="et")
                        nc.scalar.activation(et[:, :jn * 512], st[:, :jn * 512],
                                             Exp, scale=SCALE)
                        for cx in range(jn):
                            j = j0 + cx
                            h = hp * 2 + h2
                            nc.tensor.matmul(
                                pv[h2][0:65, :],
                                lhsT=vt[j][:, h * 65:(h + 1) * 65],
                                rhs=et[:, cx * 512:(cx + 1) * 512],
                                start=(j == 0), stop=(j == N // 128 - 1),
                            )
                        if gi == 2 and pb:
                            for _ in range(2):
                                if pb:
                                    proj_block(pb.pop(0))
                    srow = work.tile([1, 512], f32, tag="srow", bufs=4,
                                     name="srow")
                    nc.vector.tensor_copy(srow, pv[h2][64:65, :])
                    sinv = work.tile([1, 512], f32, tag="sinv", bufs=4,
                                     name="sinv")
                    nc.vector.reciprocal_approx_fast(sinv, srow)
                    bc = work.tile([64, 512], f32, tag="bc", bufs=4, name="bc")
                    nc.gpsimd.partition_broadcast(bc, sinv)
                    nc.vector.tensor_mul(
                        att[hp][h2 * 64:h2 * 64 + 64, qb * 512:(qb + 1) * 512],
                        pv[h2][0:64, :],
                        bc,
                    )
                for ic in pb:
                    proj_block(ic)

            # ---- qkv prefix: k(hp0), q(hp0) qb0, all v ------------------
            for nch in range(N // 512):
                qk_group(kt[0], CW, nch)
            qk_group(qt[0], 0, 0)
            for nt in range(N // 128):
                v_group(nt)

            # ---- attention, with remaining qkv between early units ------
            attn_unit(0, 0)
            for nch in range(1, N // 512):       # q(hp0) rest
                qk_group(qt[0], 0, nch)
            for nch in range(N // 512):          # k(hp1)
                qk_group(kt[1], CW + 128, nch)
            qk_group(qt[1], 128, 0)
            attn_unit(0, 1)
            for nch in range(1, N // 512):       # q(hp1) rest
                qk_group(qt[1], 128, nch)
            for nch in range(N // 512):          # k(hp2)
                qk_group(kt[2], CW + 256, nch)
            qk_group(qt[2], 256, 0)
            attn_unit(0, 2)
            for nch in range(1, N // 512):       # q(hp2) rest
                qk_group(qt[2], 256, nch)

            for qb in range(1, N // 512):
                for hp in range(HPC):
                    attn_unit(qb, hp,
                              proj_blocks=range((qb - 1) * 4, qb * 4)
                              if hp == 0 else ())
            for ic in range(12, 16):
                proj_block(ic)

    nc.compile()
    return nc


def _get_nc():
    if "nc" not in _NC_CACHE:
        _NC_CACHE["nc"] = _build()
    return _NC_CACHE["nc"]


def _ensure_ntff_hook():
    """The agent image's ``antenv`` lacks ``axon_hooks``; synthesize it so
    ``run_bass_kernel_spmd(trace=True)`` can capture NTFF profiles."""
    import types
    try:
        from antenv.axon_hooks import get_axon_ntff_profile_hook  # noqa: F401
        return
    except ImportError:
        pass
    import antenv
    from trn_agent_boot.trn_boot import _ntff_profile_via_ctypes
    hook = _ntff_profile_via_ctypes("/opt/axon/libaxon_pjrt.so")
    mod = types.ModuleType("antenv.axon_hooks")
    mod._hook = hook
    mod.get_axon_ntff_profile_hook = lambda: mod._hook

    def _set(h):
        mod._hook = h

    mod.set_axon_ntff_profile_hook = _set
    sys.modules["antenv.axon_hooks"] = mod
    antenv.axon_hooks = mod


def kernel(trace=False, **inputs):
    x = np.asarray(inputs["x"], np.float32)
    qkv_w = np.asarray(inputs["qkv_w"], np.float32)
    proj_w = np.asarray(inputs["proj_w"], np.float32)
    proj_b = np.asarray(inputs["proj_b"], np.float32)

    nc = _get_nc()

    xTb = np.ascontiguousarray(x.transpose(0, 2, 1)).astype(ml_dtypes.bfloat16)
    wqkvT = np.ascontiguousarray(qkv_w.T).astype(ml_dtypes.bfloat16)
    wprojT = np.ascontiguousarray(proj_w.T).astype(ml_dtypes.bfloat16)

    in_maps = []
    for c in range(NCORES):
        b, hg = divmod(c, 2)
        cs = hg * CW
        wslice = np.concatenate(
            [wqkvT[:, s + cs:s + cs + CW] for s in (0, C, 2 * C)], axis=1)
        in_maps.append({
            "xT": xTb[b],
            "wqkvT": np.ascontiguousarray(wslice),
            "wprojT": np.ascontiguousarray(wprojT[cs:cs + CW, :]),
        })

    from concourse import bass_utils
    if trace:
        _ensure_ntff_hook()
        bass_utils.upload_artifacts = lambda tmpdir: tmpdir
    res = bass_utils.run_bass_kernel_spmd(
        nc, in_maps, core_ids=list(range(NCORES)), trace=trace,
    )

    out = np.empty((B, N, C), np.float32)
    for b in range(B):
        out[b] = res.results[2 * b]["out"] + res.results[2 * b + 1]["out"]
    out += proj_b

    if trace:
        return out, res
    return out


# revision 8
# speedup vs baseline: 1.1567x; 1.0475x over previous
"""Multi-head attention (B=4, N=2048, C=768, H=12, Dh=64) on 8 TRN2 NeuronCores.

Sharding: tensor-parallel on heads. 2 cores per batch; each core owns 6 of the
12 heads over the FULL 2048-token sequence, so no K/V projection work is
duplicated (a query-split layout computes each batch's K/V twice; this layout
cuts per-core PE work from 614k to 541k cycles). Each core emits a partial
projection output [2048, 768] (its heads' slice of the contraction); the host
sums the two partials per batch and adds the bias as the unshard step (the
spec's "all-reduce after proj" done host-side).

Per-core inputs (partition dim first):
  xT     [768, 2048]  bf16  x[b].T
  wqkvT  [768, 1152]  bf16  [q | k | v] columns for this core's 6 heads
  wprojT [384, 768]   bf16  proj_w.T rows for this core's 6 heads
  out    [2048, 768]  f32   partial projection output

Schedule note — the chip power limiter is the binding constraint: all 8 cores
share one NeuronDevice's power envelope, and firmware clamps the PE to K=4/8
(1.2 GHz) when sustained PE duty is too high. Variants that overlap qkv or
projection work into the attention flow (raising attention-phase PE duty from
~78% toward ~95%) measured 373->477/412/373 ns*1e3 with 120-180 us spent
clamped. This bursty schedule (qkv upfront, ACT-paced attention, projection
at query-block boundaries) measures fastest. Attention is ScalarE-paced: exp
of all 25.2M logits at 153.6 G/s is ~170 us min, matmul stream is 225 us.

Pipeline:
  - qkv projection in bf16 (psum f32), all upfront; q/k stored transposed
    [d, n] packed two heads per 128-partition tile; v stored [token, d]
    augmented with a ones column per head ([64 v | 1] x 6 heads).
  - scores: S^T chunks [128 kv-rows, 512 q] = kT-slice.T @ qT-slice (K=64);
    exp on ScalarE over [128, <=1536] PSUM spans with the 1/sqrt(Dh) scale
    folded into the ACTIVATE affine (no max-subtraction; |S|<=~10 here).
  - PV: one matmul per (kv-chunk, head) with augmented V stationary [128, 65]
    -> attn-out.T rows 0..63 + softmax denominator at row 64 of the same PSUM
    accumulation group.
  - normalize: denominator row -> reciprocal_approx_fast -> gpsimd partition
    broadcast -> vector multiply -> bf16 attnT tiles (proj-ready layout).
  - projection per 512-row query block (partial d; no bias on device).
  PSUM: st-tag 2x3 banks (shared by qkv groups, scores, proj) + pv 2x1 = 8.
"""

import sys

if "/opt/trn_rl_repo" not in sys.path:
    sys.path.insert(0, "/opt/trn_rl_repo")

import numpy as np
import ml_dtypes

B, N, C = 4, 2048, 768
H, Dh = 12, 64
HC = H // 2            # heads per core
HPC = HC // 2          # head-pairs per core (3)
CW = HC * Dh           # 384 output-d columns per core
SCALE = Dh ** -0.5
CCH = C // 128         # 6 contraction chunks
NCORES = 8
JG = [(0, 3), (3, 3), (6, 3), (9, 3), (12, 2), (14, 2)]  # kv j-block groups

_NC_CACHE = {}


def _build():
    import concourse.bass as bass
    import concourse.tile as tile
    import concourse.mybir as mybir
    from concourse import bacc

    f32 = mybir.dt.float32
    bf16 = mybir.dt.bfloat16
    Exp = mybir.ActivationFunctionType.Exp

    nc = bacc.Bacc(
        "TRN2",
        target_bir_lowering=False,
        debug=False,
        enable_asserts=False,
        num_devices=NCORES,
    )

    xT = nc.dram_tensor("xT", [C, N], bf16, kind="ExternalInput").ap()
    wqkvT = nc.dram_tensor("wqkvT", [C, 3 * CW], bf16, kind="ExternalInput").ap()
    wprojT = nc.dram_tensor("wprojT", [CW, C], bf16, kind="ExternalInput").ap()
    out = nc.dram_tensor("out", [N, C], f32, kind="ExternalOutput").ap()

    with tile.TileContext(nc) as tc:
        from contextlib import ExitStack

        with ExitStack() as ctx:
            singles = ctx.enter_context(tc.tile_pool(name="singles", bufs=1))
            psum = ctx.enter_context(tc.tile_pool(name="psum", bufs=1, space="PSUM"))

            # ---- load phase-A inputs (released after qkv) ---------------
            load = tc.alloc_tile_pool(name="load", bufs=1)
            xt = [load.tile([128, N], bf16, tag=f"xt{i}", name=f"xt{i}")
                  for i in range(CCH)]
            wq = [load.tile([128, 3 * CW], bf16, tag=f"wq{i}", name=f"wq{i}")
                  for i in range(CCH)]
            for i in range(CCH):
                nc.gpsimd.dma_start(out=wq[i][:, 0:128],
                                    in_=wqkvT[i * 128:(i + 1) * 128, 0:128])
            for i in range(CCH):
                nc.gpsimd.dma_start(out=wq[i][:, 128:],
                                    in_=wqkvT[i * 128:(i + 1) * 128, 128:])
            for nch in range(N // 512):
                for i in range(CCH):
                    nc.sync.dma_start(
                        out=xt[i][:, nch * 512:(nch + 1) * 512],
                        in_=xT[i * 128:(i + 1) * 128, nch * 512:(nch + 1) * 512])
            wp = []
            for i in range(HPC):
                t = singles.tile([128, C], bf16, tag=f"wp{i}", name=f"wp{i}")
                nc.gpsimd.dma_start(out=t, in_=wprojT[i * 128:(i + 1) * 128, :])
                wp.append(t)

            # ---- qkv projections, results stored bf16 -------------------
            qt = [singles.tile([128, N], bf16, tag=f"qt{i}", name=f"qt{i}")
                  for i in range(HPC)]
            kt = [singles.tile([128, N], bf16, tag=f"kt{i}", name=f"kt{i}")
                  for i in range(HPC)]
            # v_aug: per 128-token tile, 6 heads x (64 v-cols + ones col)
            vt = [singles.tile([128, HC * 65], bf16, tag=f"vt{i}", name=f"vt{i}")
                  for i in range(N // 128)]

            # qT[d, n] and kT[d, n], two heads per 128-partition tile
            for dt in range(HPC):
                for nch in range(N // 512):
                    ps = psum.tile([128, 512], f32, tag="st", bufs=2, name="ps_q")
                    for cc in range(CCH):
                        nc.tensor.matmul(
                            ps,
                            lhsT=wq[cc][:, dt * 128:(dt + 1) * 128],
                            rhs=xt[cc][:, nch * 512:(nch + 1) * 512],
                            start=(cc == 0), stop=(cc == CCH - 1),
                        )
                    nc.vector.tensor_copy(qt[dt][:, nch * 512:(nch + 1) * 512], ps)
                for nch in range(N // 512):
                    ps = psum.tile([128, 512], f32, tag="st", bufs=2, name="ps_k")
                    for cc in range(CCH):
                        nc.tensor.matmul(
                            ps,
                            lhsT=wq[cc][:, CW + dt * 128:CW + (dt + 1) * 128],
                            rhs=xt[cc][:, nch * 512:(nch + 1) * 512],
                            start=(cc == 0), stop=(cc == CCH - 1),
                        )
                    nc.vector.tensor_copy(kt[dt][:, nch * 512:(nch + 1) * 512], ps)

            # v in [token, d] layout: v[n, d] = sum_c xT[c, n] * wv[c, d]
            for nt in range(N // 128):
                vaug = vt[nt].rearrange("p (h e) -> p h e", e=65)
                nc.vector.memset(vaug[:, :, 64:65], 1.0)
                ps = psum.tile([128, 512], f32, tag="st", bufs=2, name="ps_v")
                for cc in range(CCH):
                    nc.tensor.matmul(
                        ps[:, :CW],
                        lhsT=xt[cc][:, nt * 128:(nt + 1) * 128],
                        rhs=wq[cc][:, 2 * CW:3 * CW],
                        start=(cc == 0), stop=(cc == CCH - 1),
                    )
                nc.vector.tensor_copy(
                    vaug[:, :, 0:64],
                    ps[:, :CW].rearrange("p (h e) -> p h e", e=64),
                )

            load.release()

            # ---- attention ----------------------------------------------
            work = ctx.enter_context(tc.tile_pool(name="work", bufs=4))
            att = [singles.tile([128, N], bf16, tag=f"att{i}", name=f"att{i}")
                   for i in range(HPC)]

            for qb in range(N // 512):           # 512-wide query block
                for hp in range(HPC):            # head pair
                    pv = []
                    for h2 in range(2):
                        pv.append(psum.tile([128, 512], f32, tag="pv",
                                            bufs=2, name=f"pv{h2}"))
                    for (j0, jn) in JG:          # j-groups of up to 3x128 rows
                        for h2 in range(2):
                            hb = h2 * 64
                            st = psum.tile([128, 1536], f32, tag="st", bufs=2,
                                           name="st")
                            for cx in range(jn):
                                j = j0 + cx
                                nc.tensor.matmul(
                                    st[:, cx * 512:(cx + 1) * 512],
                                    lhsT=kt[hp][hb:hb + 64, j * 128:(j + 1) * 128],
                                    rhs=qt[hp][hb:hb + 64, qb * 512:(qb + 1) * 512],
                                    start=True, stop=True,
                                )
                            et = work.tile([128, 1536], bf16, tag="et", bufs=6,
                                           name="et")
                            nc.scalar.activation(et[:, :jn * 512],
                                                 st[:, :jn * 512],
                                                 Exp, scale=SCALE)
                            for cx in range(jn):
                                j = j0 + cx
                                h = hp * 2 + h2
                                nc.tensor.matmul(
                                    pv[h2][0:65, :],
                                    lhsT=vt[j][:, h * 65:(h + 1) * 65],
                                    rhs=et[:, cx * 512:(cx + 1) * 512],
                                    start=(j == 0), stop=(j == N // 128 - 1),
                                )
                    for h2 in range(2):
                        srow = work.tile([1, 512], f32, tag="srow",
                                         bufs=4, name="srow")
                        nc.vector.tensor_copy(srow, pv[h2][64:65, :])
                        sinv = work.tile([1, 512], f32, tag="sinv",
                                         bufs=4, name="sinv")
                        nc.vector.reciprocal_approx_fast(sinv, srow)
                        bc = work.tile([64, 512], f32, tag="bc",
                                       bufs=4, name="bc")
                        nc.gpsimd.partition_broadcast(bc, sinv)
                        nc.vector.tensor_mul(
                            att[hp][h2 * 64:h2 * 64 + 64, qb * 512:(qb + 1) * 512],
                            pv[h2][0:64, :],
                            bc,
                        )

                # ---- projection for this query block (partial d) --------
                for ic in range(qb * 4, qb * 4 + 4):
                    pj = psum.tile([128, C], f32, tag="st", bufs=2, name="pj")
                    for (d0, dw) in ((0, 512), (512, 256)):
                        for dt in range(HPC):
                            nc.tensor.matmul(
                                pj[:, d0:d0 + dw],
                                lhsT=att[dt][:, ic * 128:(ic + 1) * 128],
                                rhs=wp[dt][:, d0:d0 + dw],
                                start=(dt == 0), stop=(dt == HPC - 1),
                            )
                    osb = work.tile([128, C], f32, tag="osb", bufs=3, name="osb")
                    nc.vector.tensor_copy(osb, pj)
                    nc.sync.dma_start(out=out[ic * 128:(ic + 1) * 128, :], in_=osb)

    nc.compile()
    return nc


def _get_nc():
    if "nc" not in _NC_CACHE:
        _NC_CACHE["nc"] = _build()
    return _NC_CACHE["nc"]


def _ensure_ntff_hook():
    """The agent image's ``antenv`` lacks ``axon_hooks``; synthesize it so
    ``run_bass_kernel_spmd(trace=True)`` can capture NTFF profiles."""
    import types
    try:
        from antenv.axon_hooks import get_axon_ntff_profile_hook  # noqa: F401
        return
    except ImportError:
        pass
    import antenv
    from trn_agent_boot.trn_boot import _ntff_profile_via_ctypes
    hook = _ntff_profile_via_ctypes("/opt/axon/libaxon_pjrt.so")
    mod = types.ModuleType("antenv.axon_hooks")
    mod._hook = hook
    mod.get_axon_ntff_profile_hook = lambda: mod._hook

    def _set(h):
        mod._hook = h

    mod.set_axon_ntff_profile_hook = _set
    sys.modules["antenv.axon_hooks"] = mod
    antenv.axon_hooks = mod


def kernel(trace=False, **inputs):
    x = np.asarray(inputs["x"], np.float32)
    qkv_w = np.asarray(inputs["qkv_w"], np.float32)
    proj_w = np.asarray(inputs["proj_w"], np.float32)
    proj_b = np.asarray(inputs["proj_b"], np.float32)

    nc = _get_nc()

    xTb = np.ascontiguousarray(x.transpose(0, 2, 1)).astype(ml_dtypes.bfloat16)
    wqkvT = np.ascontiguousarray(qkv_w.T).astype(ml_dtypes.bfloat16)
    wprojT = np.ascontiguousarray(proj_w.T).astype(ml_dtypes.bfloat16)

    in_maps = []
    for c in range(NCORES):
        b, hg = divmod(c, 2)
        cs = hg * CW
        wslice = np.concatenate(
            [wqkvT[:, s + cs:s + cs + CW] for s in (0, C, 2 * C)], axis=1)
        in_maps.append({
            "xT": xTb[b],
            "wqkvT": np.ascontiguousarray(wslice),
            "wprojT": np.ascontiguousarray(wprojT[cs:cs + CW, :]),
        })

    from concourse import bass_utils
    if trace:
        _ensure_ntff_hook()
        bass_utils.upload_artifacts = lambda tmpdir: tmpdir
    res = bass_utils.run_bass_kernel_spmd(
        nc, in_maps, core_ids=list(range(NCORES)), trace=trace,
    )

    out = np.empty((B, N, C), np.float32)
    for b in range(B):
        out[b] = res.results[2 * b]["out"] + res.results[2 * b + 1]["out"]
    out += proj_b

    if trace:
        return out, res
    return out


# revision 9
# speedup vs baseline: 1.2193x; 1.0542x over previous
"""Multi-head attention (B=4, N=2048, C=768, H=12, Dh=64) on 8 TRN2 NeuronCores.

Sharding: tensor-parallel on heads. 2 cores per batch; each core owns 6 of the
12 heads over the FULL 2048-token sequence, so no K/V projection work is
duplicated (a query-split layout computes each batch's K/V twice; this layout
cuts per-core PE work from 614k to 541k cycles). Each core emits a partial
projection output [2048, 768] (its heads' slice of the contraction); the host
sums the two partials per batch and adds the bias as the unshard step (the
spec's "all-reduce after proj" done host-side).

Per-core inputs (partition dim first):
  xT     [768, 2048]  bf16  x[b].T
  wqkvT  [768, 1152]  bf16  [q | k | v] columns for this core's 6 heads
  wprojT [384, 768]   bf16  proj_w.T rows for this core's 6 heads
  out    [2048, 768]  f32   partial projection output

Schedule note — the chip power limiter is the binding constraint: all 8 cores
share one NeuronDevice's power envelope, and firmware clamps the PE to K=4/8
(1.2 GHz) when sustained PE duty is too high. Variants that overlap qkv or
projection work into the attention flow (raising attention-phase PE duty from
~78% toward ~95%) measured 373->477/412/373 ns*1e3 with 120-180 us spent
clamped. This bursty schedule (qkv upfront, ACT-paced attention, projection
at query-block boundaries) measures fastest. Attention is ScalarE-paced: exp
of all 25.2M logits at 153.6 G/s is ~170 us min, matmul stream is 225 us.

Pipeline:
  - qkv projection in bf16 (psum f32), all upfront; q/k stored transposed
    [d, n] packed two heads per 128-partition tile; v stored [token, d]
    augmented with a ones column per head ([64 v | 1] x 6 heads).
  - scores: S^T chunks [128 kv-rows, 512 q] = kT-slice.T @ qT-slice (K=64);
    exp on ScalarE over [128, <=1536] PSUM spans with the 1/sqrt(Dh) scale
    folded into the ACTIVATE affine (no max-subtraction; |S|<=~10 here).
  - PV: one matmul per (kv-chunk, head) with augmented V stationary [128, 65]
    -> attn-out.T rows 0..63 + softmax denominator at row 64 of the same PSUM
    accumulation group.
  - normalize: denominator row -> reciprocal_approx_fast -> gpsimd partition
    broadcast -> vector multiply -> bf16 attnT tiles (proj-ready layout).
  - projection per 512-row query block (partial d; no bias on device).
  PSUM: st-tag 2x3 banks (shared by qkv groups, scores, proj) + pv 2x1 = 8.
"""

import sys

if "/opt/trn_rl_repo" not in sys.path:
    sys.path.insert(0, "/opt/trn_rl_repo")

import numpy as np
import ml_dtypes

B, N, C = 4, 2048, 768
H, Dh = 12, 64
HC = H // 2            # heads per core
HPC = HC // 2          # head-pairs per core (3)
CW = HC * Dh           # 384 output-d columns per core
SCALE = Dh ** -0.5
CCH = C // 128         # 6 contraction chunks
NCORES = 8
JG = [(0, 3), (3, 3), (6, 3), (9, 3), (12, 2), (14, 2)]  # kv j-block groups

_NC_CACHE = {}


def _build():
    import concourse.bass as bass
    import concourse.tile as tile
    import concourse.mybir as mybir
    from concourse import bacc

    f32 = mybir.dt.float32
    bf16 = mybir.dt.bfloat16
    Exp = mybir.ActivationFunctionType.Exp

    nc = bacc.Bacc(
        "TRN2",
        target_bir_lowering=False,
        debug=False,
        enable_asserts=False,
        num_devices=NCORES,
    )

    xT = nc.dram_tensor("xT", [C, N], bf16, kind="ExternalInput").ap()
    wqkvT = nc.dram_tensor("wqkvT", [C, 3 * CW], bf16, kind="ExternalInput").ap()
    wprojT = nc.dram_tensor("wprojT", [CW, C], bf16, kind="ExternalInput").ap()
    out = nc.dram_tensor("out", [N, C], f32, kind="ExternalOutput").ap()

    with tile.TileContext(nc) as tc:
        from contextlib import ExitStack

        with ExitStack() as ctx:
            singles = ctx.enter_context(tc.tile_pool(name="singles", bufs=1))
            psum = ctx.enter_context(tc.tile_pool(name="psum", bufs=1, space="PSUM"))

            # ---- load phase-A inputs (released after qkv) ---------------
            load = tc.alloc_tile_pool(name="load", bufs=1)
            xt = [load.tile([128, N], bf16, tag=f"xt{i}", name=f"xt{i}")
                  for i in range(CCH)]
            wq = [load.tile([128, 3 * CW], bf16, tag=f"wq{i}", name=f"wq{i}")
                  for i in range(CCH)]
            for i in range(CCH):
                nc.gpsimd.dma_start(out=wq[i][:, 0:128],
                                    in_=wqkvT[i * 128:(i + 1) * 128, 0:128])
            for i in range(CCH):
                nc.gpsimd.dma_start(out=wq[i][:, 128:],
                                    in_=wqkvT[i * 128:(i + 1) * 128, 128:])
            for nch in range(N // 512):
                for i in range(CCH):
                    nc.sync.dma_start(
                        out=xt[i][:, nch * 512:(nch + 1) * 512],
                        in_=xT[i * 128:(i + 1) * 128, nch * 512:(nch + 1) * 512])
            wp = []
            for i in range(HPC):
                t = singles.tile([128, C], bf16, tag=f"wp{i}", name=f"wp{i}")
                nc.gpsimd.dma_start(out=t, in_=wprojT[i * 128:(i + 1) * 128, :])
                wp.append(t)

            # ---- qkv projections, results stored bf16 -------------------
            qt = [singles.tile([128, N], bf16, tag=f"qt{i}", name=f"qt{i}")
                  for i in range(HPC)]
            kt = [singles.tile([128, N], bf16, tag=f"kt{i}", name=f"kt{i}")
                  for i in range(HPC)]
            # v_aug: per 128-token tile, 6 heads x (64 v-cols + ones col)
            vt = [singles.tile([128, HC * 65], bf16, tag=f"vt{i}", name=f"vt{i}")
                  for i in range(N // 128)]

            # qT[d, n] and kT[d, n], two heads per 128-partition tile
            for dt in range(HPC):
                for nch in range(N // 512):
                    ps = psum.tile([128, 512], f32, tag="st", bufs=2, name="ps_q")
                    for cc in range(CCH):
                        nc.tensor.matmul(
                            ps,
                            lhsT=wq[cc][:, dt * 128:(dt + 1) * 128],
                            rhs=xt[cc][:, nch * 512:(nch + 1) * 512],
                            start=(cc == 0), stop=(cc == CCH - 1),
                        )
                    nc.vector.tensor_copy(qt[dt][:, nch * 512:(nch + 1) * 512], ps)
                for nch in range(N // 512):
                    ps = psum.tile([128, 512], f32, tag="st", bufs=2, name="ps_k")
                    for cc in range(CCH):
                        nc.tensor.matmul(
                            ps,
                            lhsT=wq[cc][:, CW + dt * 128:CW + (dt + 1) * 128],
                            rhs=xt[cc][:, nch * 512:(nch + 1) * 512],
                            start=(cc == 0), stop=(cc == CCH - 1),
                        )
                    nc.vector.tensor_copy(kt[dt][:, nch * 512:(nch + 1) * 512], ps)

            # v in [token, d] layout: v[n, d] = sum_c xT[c, n] * wv[c, d]
            for nt in range(N // 128):
                vaug = vt[nt].rearrange("p (h e) -> p h e", e=65)
                nc.vector.memset(vaug[:, :, 64:65], 1.0)
                ps = psum.tile([128, 512], f32, tag="st", bufs=2, name="ps_v")
                for cc in range(CCH):
                    nc.tensor.matmul(
                        ps[:, :CW],
                        lhsT=xt[cc][:, nt * 128:(nt + 1) * 128],
                        rhs=wq[cc][:, 2 * CW:3 * CW],
                        start=(cc == 0), stop=(cc == CCH - 1),
                    )
                nc.vector.tensor_copy(
                    vaug[:, :, 0:64],
                    ps[:, :CW].rearrange("p (h e) -> p h e", e=64),
                )

            load.release()

            # ---- attention ----------------------------------------------
            work = ctx.enter_context(tc.tile_pool(name="work", bufs=4))
            att = [singles.tile([128, N], bf16, tag=f"att{i}", name=f"att{i}")
                   for i in range(HPC)]

            for qb in range(N // 512):           # 512-wide query block
                for hp in range(HPC):            # head pair
                    pv = []
                    for h2 in range(2):
                        pv.append(psum.tile([128, 512], f32, tag="pv",
                                            bufs=2, name=f"pv{h2}"))
                    # the two heads run as separate streams so each head's
                    # normalize chain overlaps the other head's stream
                    # instead of stalling the next unit's PV accumulation
                    for h2 in range(2):
                        hb = h2 * 64
                        for (j0, jn) in JG:      # j-groups of up to 3x128 rows
                            st = psum.tile([128, 1536], f32, tag="st", bufs=2,
                                           name="st")
                            for cx in range(jn):
                                j = j0 + cx
                                nc.tensor.matmul(
                                    st[:, cx * 512:(cx + 1) * 512],
                                    lhsT=kt[hp][hb:hb + 64, j * 128:(j + 1) * 128],
                                    rhs=qt[hp][hb:hb + 64, qb * 512:(qb + 1) * 512],
                                    start=True, stop=True,
                                )
                            et = work.tile([128, 1536], bf16, tag="et", bufs=8,
                                           name="et")
                            nc.scalar.activation(et[:, :jn * 512],
                                                 st[:, :jn * 512],
                                                 Exp, scale=SCALE)
                            for cx in range(jn):
                                j = j0 + cx
                                h = hp * 2 + h2
                                nc.tensor.matmul(
                                    pv[h2][0:65, :],
                                    lhsT=vt[j][:, h * 65:(h + 1) * 65],
                                    rhs=et[:, cx * 512:(cx + 1) * 512],
                                    start=(j == 0), stop=(j == N // 128 - 1),
                                )
                        srow = work.tile([1, 512], f32, tag="srow",
                                         bufs=4, name="srow")
                        nc.vector.tensor_copy(srow, pv[h2][64:65, :])
                        sinv = work.tile([1, 512], f32, tag="sinv",
                                         bufs=4, name="sinv")
                        nc.vector.reciprocal_approx_fast(sinv, srow)
                        bc = work.tile([64, 512], f32, tag="bc",
                                       bufs=4, name="bc")
                        nc.gpsimd.partition_broadcast(bc, sinv)
                        nc.vector.tensor_mul(
                            att[hp][h2 * 64:h2 * 64 + 64, qb * 512:(qb + 1) * 512],
                            pv[h2][0:64, :],
                            bc,
                        )

                # ---- projection for this query block (partial d) --------
                for ic in range(qb * 4, qb * 4 + 4):
                    pj = psum.tile([128, C], f32, tag="st", bufs=2, name="pj")
                    for (d0, dw) in ((0, 512), (512, 256)):
                        for dt in range(HPC):
                            nc.tensor.matmul(
                                pj[:, d0:d0 + dw],
                                lhsT=att[dt][:, ic * 128:(ic + 1) * 128],
                                rhs=wp[dt][:, d0:d0 + dw],
                                start=(dt == 0), stop=(dt == HPC - 1),
                            )
                    osb = work.tile([128, C], f32, tag="osb", bufs=3, name="osb")
                    nc.vector.tensor_copy(osb, pj)
                    nc.sync.dma_start(out=out[ic * 128:(ic + 1) * 128, :], in_=osb)

    nc.compile()
    return nc


def _get_nc():
    if "nc" not in _NC_CACHE:
        _NC_CACHE["nc"] = _build()
    return _NC_CACHE["nc"]


def _ensure_ntff_hook():
    """The agent image's ``antenv`` lacks ``axon_hooks``; synthesize it so
    ``run_bass_kernel_spmd(trace=True)`` can capture NTFF profiles."""
    import types
    try:
        from antenv.axon_hooks import get_axon_ntff_profile_hook  # noqa: F401
        return
    except ImportError:
        pass
    import antenv
    from trn_agent_boot.trn_boot import _ntff_profile_via_ctypes
    hook = _ntff_profile_via_ctypes("/opt/axon/libaxon_pjrt.so")
    mod = types.ModuleType("antenv.axon_hooks")
    mod._hook = hook
    mod.get_axon_ntff_profile_hook = lambda: mod._hook

    def _set(h):
        mod._hook = h

    mod.set_axon_ntff_profile_hook = _set
    sys.modules["antenv.axon_hooks"] = mod
    antenv.axon_hooks = mod


def kernel(trace=False, **inputs):
    x = np.asarray(inputs["x"], np.float32)
    qkv_w = np.asarray(inputs["qkv_w"], np.float32)
    proj_w = np.asarray(inputs["proj_w"], np.float32)
    proj_b = np.asarray(inputs["proj_b"], np.float32)

    nc = _get_nc()

    xTb = np.ascontiguousarray(x.transpose(0, 2, 1)).astype(ml_dtypes.bfloat16)
    wqkvT = np.ascontiguousarray(qkv_w.T).astype(ml_dtypes.bfloat16)
    wprojT = np.ascontiguousarray(proj_w.T).astype(ml_dtypes.bfloat16)

    in_maps = []
    for c in range(NCORES):
        b, hg = divmod(c, 2)
        cs = hg * CW
        wslice = np.concatenate(
            [wqkvT[:, s + cs:s + cs + CW] for s in (0, C, 2 * C)], axis=1)
        in_maps.append({
            "xT": xTb[b],
            "wqkvT": np.ascontiguousarray(wslice),
            "wprojT": np.ascontiguousarray(wprojT[cs:cs + CW, :]),
        })

    from concourse import bass_utils
    if trace:
        _ensure_ntff_hook()
        bass_utils.upload_artifacts = lambda tmpdir: tmpdir
    res = bass_utils.run_bass_kernel_spmd(
        nc, in_maps, core_ids=list(range(NCORES)), trace=trace,
    )

    out = np.empty((B, N, C), np.float32)
    for b in range(B):
        out[b] = res.results[2 * b]["out"] + res.results[2 * b + 1]["out"]
    out += proj_b

    if trace:
        return out, res
    return out


# revision 17
# speedup vs baseline: 1.2610x; 1.0342x over previous
"""Multi-head attention (B=4, N=2048, C=768, H=12, Dh=64) on 8 TRN2 NeuronCores.

Sharding: tensor-parallel on heads. 2 cores per batch; each core owns 6 of the
12 heads over the FULL 2048-token sequence, so no K/V projection work is
duplicated (a query-split layout computes each batch's K/V twice; this layout
cuts per-core PE work from 614k to 541k cycles). Each core emits a partial
projection output [2048, 768] (its heads' slice of the contraction); the host
sums the two partials per batch and adds the bias as the unshard step (the
spec's "all-reduce after proj" done host-side).

Per-core inputs (partition dim first):
  xT     [768, 2048]  bf16  x[b].T
  wqkvT  [768, 1152]  bf16  [q | k | v] columns for this core's 6 heads
  wprojT [384, 768]   bf16  proj_w.T rows for this core's 6 heads
  out    [2048, 768]  f32   partial projection output

Schedule note — the chip power limiter is the binding constraint: all 8 cores
share one NeuronDevice's power envelope, and firmware clamps the PE to K=4/8
(1.2 GHz) when sustained PE duty is too high. Variants that overlap qkv or
projection work into the attention flow (raising attention-phase PE duty from
~78% toward ~95%) measured 373->477/412/373 ns*1e3 with 120-180 us spent
clamped. This bursty schedule (qkv upfront, ACT-paced attention, projection
at query-block boundaries) measures fastest. Attention is ScalarE-paced: exp
of all 25.2M logits at 153.6 G/s is ~170 us min, matmul stream is 225 us.

Pipeline:
  - qkv projection in bf16 (psum f32), all upfront; q/k stored transposed
    [d, n] packed two heads per 128-partition tile; v stored [token, d]
    augmented with a ones column per head ([64 v | 1] x 6 heads).
  - scores: S^T chunks [128 kv-rows, 512 q] = kT-slice.T @ qT-slice (K=64);
    exp on ScalarE over [128, <=1536] PSUM spans with the 1/sqrt(Dh) scale
    folded into the ACTIVATE affine (no max-subtraction; |S|<=~10 here).
  - PV: one matmul per (kv-chunk, head) with augmented V stationary [128, 65]
    -> attn-out.T rows 0..63 + softmax denominator at row 64 of the same PSUM
    accumulation group.
  - normalize: denominator row -> reciprocal_approx_fast -> gpsimd partition
    broadcast -> vector multiply -> bf16 attnT tiles (proj-ready layout).
  - projection per 512-row query block (partial d; no bias on device).
  PSUM: st-tag 2x3 banks (shared by qkv groups, scores, proj) + pv 2x1 = 8.
"""

import sys

if "/opt/trn_rl_repo" not in sys.path:
    sys.path.insert(0, "/opt/trn_rl_repo")

import numpy as np
import ml_dtypes

B, N, C = 4, 2048, 768
H, Dh = 12, 64
HC = H // 2            # heads per core
HPC = HC // 2          # head-pairs per core (3)
CW = HC * Dh           # 384 output-d columns per core
SCALE = Dh ** -0.5
CCH = C // 128         # 6 contraction chunks
NCORES = 8
JG = [(0, 3), (3, 3), (6, 3), (9, 3), (12, 2), (14, 2)]  # kv j-block groups

_NC_CACHE = {}


def _build():
    import concourse.bass as bass
    import concourse.tile as tile
    import concourse.mybir as mybir
    from concourse import bacc

    f32 = mybir.dt.float32
    bf16 = mybir.dt.bfloat16
    Exp = mybir.ActivationFunctionType.Exp

    nc = bacc.Bacc(
        "TRN2",
        target_bir_lowering=False,
        debug=False,
        enable_asserts=False,
        num_devices=NCORES,
    )

    xT = nc.dram_tensor("xT", [C, N], bf16, kind="ExternalInput").ap()
    wqkvT = nc.dram_tensor("wqkvT", [C, 3 * CW], bf16, kind="ExternalInput").ap()
    wprojT = nc.dram_tensor("wprojT", [CW, C], bf16, kind="ExternalInput").ap()
    out = nc.dram_tensor("out", [N, C], f32, kind="ExternalOutput").ap()

    with tile.TileContext(nc) as tc:
        from contextlib import ExitStack

        with ExitStack() as ctx:
            singles = ctx.enter_context(tc.tile_pool(name="singles", bufs=1))
            psum = ctx.enter_context(tc.tile_pool(name="psum", bufs=1, space="PSUM"))

            # ---- input tiles (resident all kernel: deferred q-groups
            # read xt/wq during the attention phase) ----------------------
            xt = [singles.tile([128, N], bf16, tag=f"xt{i}", name=f"xt{i}")
                  for i in range(CCH)]
            wq = [singles.tile([128, 3 * CW], bf16, tag=f"wq{i}", name=f"wq{i}")
                  for i in range(CCH)]
            # k(hp0) weight columns first: the first real PE group is k(hp0)
            for i in range(CCH):
                nc.gpsimd.dma_start(out=wq[i][:, CW:CW + 128],
                                    in_=wqkvT[i * 128:(i + 1) * 128, CW:CW + 128])
            for i in range(CCH):
                nc.gpsimd.dma_start(out=wq[i][:, 0:CW],
                                    in_=wqkvT[i * 128:(i + 1) * 128, 0:CW])
            for i in range(CCH):
                nc.gpsimd.dma_start(out=wq[i][:, CW + 128:],
                                    in_=wqkvT[i * 128:(i + 1) * 128, CW + 128:])
            for nch in range(N // 512):
                for i in range(CCH):
                    nc.sync.dma_start(
                        out=xt[i][:, nch * 512:(nch + 1) * 512],
                        in_=xT[i * 128:(i + 1) * 128, nch * 512:(nch + 1) * 512])
            wp = []
            for i in range(HPC):
                t = singles.tile([128, C], bf16, tag=f"wp{i}", name=f"wp{i}")
                nc.gpsimd.dma_start(out=t, in_=wprojT[i * 128:(i + 1) * 128, :])
                wp.append(t)

            # ---- PE warmup: HAM un-throttles after ~3.4 us of sustained
            # activity; dummy matmuls on a memset tile (no DMA dependency)
            # warm the clock gate before the real qkv stream arrives -------
            warm = singles.tile([128, 512], bf16, tag="warm", name="warm")
            nc.vector.memset(warm, 0.0)
            for _ in range(18):
                wps = psum.tile([128, 512], f32, tag="pv", bufs=2,
                                name="warmps")
                nc.tensor.matmul(wps, lhsT=warm[:, 0:128], rhs=warm,
                                 start=True, stop=True)

            # ---- qkv projections, results stored bf16 -------------------
            qt = [singles.tile([128, N], bf16, tag=f"qt{i}", name=f"qt{i}")
                  for i in range(HPC)]
            kt = [singles.tile([128, N], bf16, tag=f"kt{i}", name=f"kt{i}")
                  for i in range(HPC)]
            # v_aug: per 128-token tile, 6 heads x (64 v-cols + ones col)
            vt = [singles.tile([128, HC * 65], bf16, tag=f"vt{i}", name=f"vt{i}")
                  for i in range(N // 128)]

            def qk_group(dst, col0, nch):
                """One psum group of a q/k projection: 512 tokens x 128 d."""
                ps = psum.tile([128, 512], f32, tag="st", bufs=2, name="ps_qk")
                for cc in range(CCH):
                    nc.tensor.matmul(
                        ps,
                        lhsT=wq[cc][:, col0:col0 + 128],
                        rhs=xt[cc][:, nch * 512:(nch + 1) * 512],
                        start=(cc == 0), stop=(cc == CCH - 1),
                    )
                nc.vector.tensor_copy(dst[:, nch * 512:(nch + 1) * 512], ps)

            # prefix: k for all head-pairs, q for query-block 0 only, all v.
            # q for query-block qb+1 is emitted at the tail of unit (qb, 2),
            # where the et-buffer exp backlog keeps ScalarE busy — this
            # shortens the serial pre-attention phase by ~14 us.
            for dt in range(HPC):
                for nch in range(N // 512):
                    qk_group(kt[dt], CW + dt * 128, nch)
                qk_group(qt[dt], dt * 128, 0)

            # v in [token, d] layout: v[n, d] = sum_c xT[c, n] * wv[c, d]
            for nt in range(N // 128):
                vaug = vt[nt].rearrange("p (h e) -> p h e", e=65)
                nc.vector.memset(vaug[:, :, 64:65], 1.0)
                ps = psum.tile([128, 512], f32, tag="st", bufs=2, name="ps_v")
                for cc in range(CCH):
                    nc.tensor.matmul(
                        ps[:, :CW],
                        lhsT=xt[cc][:, nt * 128:(nt + 1) * 128],
                        rhs=wq[cc][:, 2 * CW:3 * CW],
                        start=(cc == 0), stop=(cc == CCH - 1),
                    )
                nc.vector.tensor_copy(
                    vaug[:, :, 0:64],
                    ps[:, :CW].rearrange("p (h e) -> p h e", e=64),
                )

            # ---- attention ----------------------------------------------
            work = ctx.enter_context(tc.tile_pool(name="work", bufs=4))
            att = [singles.tile([128, N], bf16, tag=f"att{i}", name=f"att{i}")
                   for i in range(HPC)]

            def proj_block(ic):
                """Projection for one 128-row block (partial: this core's d)."""
                pj = psum.tile([128, C], f32, tag="st", bufs=2, name="pj")
                for (d0, dw) in ((0, 512), (512, 256)):
                    for dt in range(HPC):
                        nc.tensor.matmul(
                            pj[:, d0:d0 + dw],
                            lhsT=att[dt][:, ic * 128:(ic + 1) * 128],
                            rhs=wp[dt][:, d0:d0 + dw],
                            start=(dt == 0), stop=(dt == HPC - 1),
                        )
                osb = work.tile([128, C], f32, tag="osb", bufs=3, name="osb")
                nc.vector.tensor_copy(osb, pj)
                nc.sync.dma_start(out=out[ic * 128:(ic + 1) * 128, :], in_=osb)

            def attn_unit(qb, hp, proj_blocks=()):
                """Scores+exp+PV+normalize, one 512-q block x head pair.

                The two heads run as separate streams so each head's
                normalize chain overlaps the other head's stream. When
                proj_blocks is set (previous query block's projection), the
                h2=0 stream emits all scores+exps first, then the proj
                matmuls, then the deferred PVs — so ScalarE chews the queued
                exp backlog instead of idling while the PE projects.
                """
                pv = [psum.tile([128, 512], f32, tag="pv", bufs=2,
                                name=f"pv{h2}") for h2 in range(2)]
                for h2 in range(2):
                    hb = h2 * 64
                    defer = h2 == 0 and proj_blocks
                    pvq = []
                    for (j0, jn) in JG:          # j-groups of up to 3x128 rows
                        st = psum.tile([128, 1536], f32, tag="st", bufs=2,
                                       name="st")
                        for cx in range(jn):
                            j = j0 + cx
                            nc.tensor.matmul(
                                st[:, cx * 512:(cx + 1) * 512],
                                lhsT=kt[hp][hb:hb + 64, j * 128:(j + 1) * 128],
                                rhs=qt[hp][hb:hb + 64, qb * 512:(qb + 1) * 512],
                                start=True, stop=True,
                            )
                        et = work.tile([128, 1536], bf16, tag="et", bufs=10,
                                       name="et")
                        nc.scalar.activation(et[:, :jn * 512], st[:, :jn * 512],
                                             Exp, scale=SCALE)
                        pvq.append((et, j0, jn))
                        if not defer:
                            for (ete, pj0, pjn) in pvq:
                                for cx in range(pjn):
                                    j = pj0 + cx
                                    h = hp * 2 + h2
                                    nc.tensor.matmul(
                                        pv[h2][0:65, :],
                                        lhsT=vt[j][:, h * 65:(h + 1) * 65],
                                        rhs=ete[:, cx * 512:(cx + 1) * 512],
                                        start=(j == 0),
                                        stop=(j == N // 128 - 1),
                                    )
                            pvq = []
                    if defer:
                        for ic in proj_blocks:
                            proj_block(ic)
                        for (ete, pj0, pjn) in pvq:
                            for cx in range(pjn):
                                j = pj0 + cx
                                h = hp * 2
                                nc.tensor.matmul(
                                    pv[0][0:65, :],
                                    lhsT=vt[j][:, h * 65:(h + 1) * 65],
                                    rhs=ete[:, cx * 512:(cx + 1) * 512],
                                    start=(j == 0), stop=(j == N // 128 - 1),
                                )
                    srow = work.tile([1, 512], f32, tag="srow", bufs=4,
                                     name="srow")
                    nc.vector.tensor_copy(srow, pv[h2][64:65, :])
                    sinv = work.tile([1, 512], f32, tag="sinv", bufs=4,
                                     name="sinv")
                    nc.vector.reciprocal_approx_fast(sinv, srow)
                    bc = work.tile([64, 512], f32, tag="bc", bufs=4, name="bc")
                    nc.gpsimd.partition_broadcast(bc, sinv)
                    nc.vector.tensor_mul(
                        att[hp][h2 * 64:h2 * 64 + 64, qb * 512:(qb + 1) * 512],
                        pv[h2][0:64, :],
                        bc,
                    )

            for qb in range(N // 512):           # 512-wide query block
                for hp in range(HPC):            # head pair
                    attn_unit(qb, hp,
                              proj_blocks=range((qb - 1) * 4, qb * 4)
                              if (hp == 0 and qb > 0) else ())
                if qb + 1 < N // 512:            # q for the next query block
                    for dt in range(HPC):
                        qk_group(qt[dt], dt * 128, qb + 1)
            for ic in range(12, 16):
                proj_block(ic)

    nc.compile()
    return nc


def _get_nc():
    if "nc" not in _NC_CACHE:
        _NC_CACHE["nc"] = _build()
    return _NC_CACHE["nc"]


def _ensure_ntff_hook():
    """The agent image's ``antenv`` lacks ``axon_hooks``; synthesize it so
    ``run_bass_kernel_spmd(trace=True)`` can capture NTFF profiles."""
    import types
    try:
        from antenv.axon_hooks import get_axon_ntff_profile_hook  # noqa: F401
        return
    except ImportError:
        pass
    import antenv
    from trn_agent_boot.trn_boot import _ntff_profile_via_ctypes
    hook = _ntff_profile_via_ctypes("/opt/axon/libaxon_pjrt.so")
    mod = types.ModuleType("antenv.axon_hooks")
    mod._hook = hook
    mod.get_axon_ntff_profile_hook = lambda: mod._hook

    def _set(h):
        mod._hook = h

    mod.set_axon_ntff_profile_hook = _set
    sys.modules["antenv.axon_hooks"] = mod
    antenv.axon_hooks = mod


def kernel(trace=False, **inputs):
    x = np.asarray(inputs["x"], np.float32)
    qkv_w = np.asarray(inputs["qkv_w"], np.float32)
    proj_w = np.asarray(inputs["proj_w"], np.float32)
    proj_b = np.asarray(inputs["proj_b"], np.float32)

    nc = _get_nc()

    xTb = np.ascontiguousarray(x.transpose(0, 2, 1)).astype(ml_dtypes.bfloat16)
    wqkvT = np.ascontiguousarray(qkv_w.T).astype(ml_dtypes.bfloat16)
    wprojT = np.ascontiguousarray(proj_w.T).astype(ml_dtypes.bfloat16)

    in_maps = []
    for c in range(NCORES):
        b, hg = divmod(c, 2)
        cs = hg * CW
        wslice = np.concatenate(
            [wqkvT[:, s + cs:s + cs + CW] for s in (0, C, 2 * C)], axis=1)
        in_maps.append({
            "xT": xTb[b],
            "wqkvT": np.ascontiguousarray(wslice),
            "wprojT": np.ascontiguousarray(wprojT[cs:cs + CW, :]),
        })

    from concourse import bass_utils
    if trace:
        _ensure_ntff_hook()
        bass_utils.upload_artifacts = lambda tmpdir: tmpdir
    res = bass_utils.run_bass_kernel_spmd(
        nc, in_maps, core_ids=list(range(NCORES)), trace=trace,
    )

    out = np.empty((B, N, C), np.float32)
    for b in range(B):
        out[b] = res.results[2 * b]["out"] + res.results[2 * b + 1]["out"]
    out += proj_b

    if trace:
        return out, res
    return out


# revision 18
# speedup vs baseline: 1.2947x; 1.0267x over previous
"""Multi-head attention (B=4, N=2048, C=768, H=12, Dh=64) on 8 TRN2 NeuronCores.

Sharding: tensor-parallel on heads. 2 cores per batch; each core owns 6 of the
12 heads over the FULL 2048-token sequence, so no K/V projection work is
duplicated (a query-split layout computes each batch's K/V twice; this layout
cuts per-core PE work from 614k to 541k cycles). Each core emits a partial
projection output [2048, 768] (its heads' slice of the contraction); the host
sums the two partials per batch and adds the bias as the unshard step (the
spec's "all-reduce after proj" done host-side).

Per-core inputs (partition dim first):
  xT     [768, 2048]  bf16  x[b].T
  wqkvT  [768, 1152]  bf16  [q | k | v] columns for this core's 6 heads
  wprojT [384, 768]   bf16  proj_w.T rows for this core's 6 heads
  out    [2048, 768]  f32   partial projection output

Schedule note — the chip power limiter is the binding constraint: all 8 cores
share one NeuronDevice's power envelope, and firmware clamps the PE to K=4/8
(1.2 GHz) when sustained PE duty is too high. Variants that overlap qkv or
projection work into the attention flow (raising attention-phase PE duty from
~78% toward ~95%) measured 373->477/412/373 ns*1e3 with 120-180 us spent
clamped. This bursty schedule (qkv upfront, ACT-paced attention, projection
at query-block boundaries) measures fastest. Attention is ScalarE-paced: exp
of all 25.2M logits at 153.6 G/s is ~170 us min, matmul stream is 225 us.

Pipeline:
  - qkv projection in bf16 (psum f32), all upfront; q/k stored transposed
    [d, n] packed two heads per 128-partition tile; v stored [token, d]
    augmented with a ones column per head ([64 v | 1] x 6 heads).
  - scores: S^T chunks [128 kv-rows, 512 q] = kT-slice.T @ qT-slice (K=64);
    exp on ScalarE over [128, <=1536] PSUM spans with the 1/sqrt(Dh) scale
    folded into the ACTIVATE affine (no max-subtraction; |S|<=~10 here).
  - PV: one matmul per (kv-chunk, head) with augmented V stationary [128, 65]
    -> attn-out.T rows 0..63 + softmax denominator at row 64 of the same PSUM
    accumulation group.
  - normalize: denominator row -> reciprocal_approx_fast -> gpsimd partition
    broadcast -> vector multiply -> bf16 attnT tiles (proj-ready layout).
  - projection per 512-row query block (partial d; no bias on device).
  PSUM: st-tag 2x3 banks (shared by qkv groups, scores, proj) + pv 2x1 = 8.
"""

import sys

if "/opt/trn_rl_repo" not in sys.path:
    sys.path.insert(0, "/opt/trn_rl_repo")

import numpy as np
import ml_dtypes

B, N, C = 4, 2048, 768
H, Dh = 12, 64
HC = H // 2            # heads per core
HPC = HC // 2          # head-pairs per core (3)
CW = HC * Dh           # 384 output-d columns per core
SCALE = Dh ** -0.5
CCH = C // 128         # 6 contraction chunks
NCORES = 8
JG = [(0, 3), (3, 3), (6, 3), (9, 3), (12, 2), (14, 2)]  # kv j-block groups

_NC_CACHE = {}


def _build():
    import concourse.bass as bass
    import concourse.tile as tile
    import concourse.mybir as mybir
    from concourse import bacc

    f32 = mybir.dt.float32
    bf16 = mybir.dt.bfloat16
    Exp = mybir.ActivationFunctionType.Exp

    nc = bacc.Bacc(
        "TRN2",
        target_bir_lowering=False,
        debug=False,
        enable_asserts=False,
        num_devices=NCORES,
    )

    xT = nc.dram_tensor("xT", [C, N], bf16, kind="ExternalInput").ap()
    wqkvT = nc.dram_tensor("wqkvT", [C, 3 * CW], bf16, kind="ExternalInput").ap()
    wprojT = nc.dram_tensor("wprojT", [CW, C], bf16, kind="ExternalInput").ap()
    out = nc.dram_tensor("out", [N, C], f32, kind="ExternalOutput").ap()

    with tile.TileContext(nc) as tc:
        from contextlib import ExitStack

        with ExitStack() as ctx:
            singles = ctx.enter_context(tc.tile_pool(name="singles", bufs=1))
            psum = ctx.enter_context(tc.tile_pool(name="psum", bufs=1, space="PSUM"))

            # ---- load phase-A inputs (released after qkv) ---------------
            load = tc.alloc_tile_pool(name="load", bufs=1)
            xt = [load.tile([128, N], bf16, tag=f"xt{i}", name=f"xt{i}")
                  for i in range(CCH)]
            wq = [load.tile([128, 3 * CW], bf16, tag=f"wq{i}", name=f"wq{i}")
                  for i in range(CCH)]
            for i in range(CCH):
                nc.gpsimd.dma_start(out=wq[i][:, 0:128],
                                    in_=wqkvT[i * 128:(i + 1) * 128, 0:128])
            for i in range(CCH):
                nc.gpsimd.dma_start(out=wq[i][:, 128:],
                                    in_=wqkvT[i * 128:(i + 1) * 128, 128:])
            for nch in range(N // 512):
                for i in range(CCH):
                    nc.sync.dma_start(
                        out=xt[i][:, nch * 512:(nch + 1) * 512],
                        in_=xT[i * 128:(i + 1) * 128, nch * 512:(nch + 1) * 512])
            wp = []
            for i in range(HPC):
                t = singles.tile([128, C], bf16, tag=f"wp{i}", name=f"wp{i}")
                nc.gpsimd.dma_start(out=t, in_=wprojT[i * 128:(i + 1) * 128, :])
                wp.append(t)

            # ---- PE warmup: HAM un-throttles after ~3.4 us of activity;
            # dummy matmuls on a memset tile (no DMA dependency) keep the
            # clock gate warm across the DMA window so the real qkv stream
            # starts at 2.4 GHz ------------------------------------------
            warm = singles.tile([128, 512], bf16, tag="warm", name="warm")
            nc.vector.memset(warm, 0.0)
            for _ in range(30):
                wps = psum.tile([128, 512], f32, tag="pv", bufs=2,
                                name="warmps")
                nc.tensor.matmul(wps, lhsT=warm[:, 0:128], rhs=warm,
                                 start=True, stop=True)

            # ---- qkv projections, results stored bf16 -------------------
            qt = [singles.tile([128, N], bf16, tag=f"qt{i}", name=f"qt{i}")
                  for i in range(HPC)]
            kt = [singles.tile([128, N], bf16, tag=f"kt{i}", name=f"kt{i}")
                  for i in range(HPC)]
            # v_aug: per 128-token tile, 6 heads x (64 v-cols + ones col)
            vt = [singles.tile([128, HC * 65], bf16, tag=f"vt{i}", name=f"vt{i}")
                  for i in range(N // 128)]

            # qT[d, n] and kT[d, n], two heads per 128-partition tile
            for dt in range(HPC):
                for nch in range(N // 512):
                    ps = psum.tile([128, 512], f32, tag="st", bufs=2, name="ps_q")
                    for cc in range(CCH):
                        nc.tensor.matmul(
                            ps,
                            lhsT=wq[cc][:, dt * 128:(dt + 1) * 128],
                            rhs=xt[cc][:, nch * 512:(nch + 1) * 512],
                            start=(cc == 0), stop=(cc == CCH - 1),
                        )
                    nc.vector.tensor_copy(qt[dt][:, nch * 512:(nch + 1) * 512], ps)
                for nch in range(N // 512):
                    ps = psum.tile([128, 512], f32, tag="st", bufs=2, name="ps_k")
                    for cc in range(CCH):
                        nc.tensor.matmul(
                            ps,
                            lhsT=wq[cc][:, CW + dt * 128:CW + (dt + 1) * 128],
                            rhs=xt[cc][:, nch * 512:(nch + 1) * 512],
                            start=(cc == 0), stop=(cc == CCH - 1),
                        )
                    nc.vector.tensor_copy(kt[dt][:, nch * 512:(nch + 1) * 512], ps)

            # v in [token, d] layout: v[n, d] = sum_c xT[c, n] * wv[c, d]
            for nt in range(N // 128):
                vaug = vt[nt].rearrange("p (h e) -> p h e", e=65)
                nc.vector.memset(vaug[:, :, 64:65], 1.0)
                ps = psum.tile([128, 512], f32, tag="st", bufs=2, name="ps_v")
                for cc in range(CCH):
                    nc.tensor.matmul(
                        ps[:, :CW],
                        lhsT=xt[cc][:, nt * 128:(nt + 1) * 128],
                        rhs=wq[cc][:, 2 * CW:3 * CW],
                        start=(cc == 0), stop=(cc == CCH - 1),
                    )
                nc.vector.tensor_copy(
                    vaug[:, :, 0:64],
                    ps[:, :CW].rearrange("p (h e) -> p h e", e=64),
                )

            load.release()

            # ---- attention ----------------------------------------------
            work = ctx.enter_context(tc.tile_pool(name="work", bufs=4))
            att = [singles.tile([128, N], bf16, tag=f"att{i}", name=f"att{i}")
                   for i in range(HPC)]

            def proj_block(ic):
                """Projection for one 128-row block (partial: this core's d)."""
                pj = psum.tile([128, C], f32, tag="st", bufs=2, name="pj")
                for (d0, dw) in ((0, 512), (512, 256)):
                    for dt in range(HPC):
                        nc.tensor.matmul(
                            pj[:, d0:d0 + dw],
                            lhsT=att[dt][:, ic * 128:(ic + 1) * 128],
                            rhs=wp[dt][:, d0:d0 + dw],
                            start=(dt == 0), stop=(dt == HPC - 1),
                        )
                osb = work.tile([128, C], f32, tag="osb", bufs=3, name="osb")
                nc.vector.tensor_copy(osb, pj)
                nc.sync.dma_start(out=out[ic * 128:(ic + 1) * 128, :], in_=osb)

            def attn_unit(qb, hp, proj_blocks=()):
                """Scores+exp+PV+normalize, one 512-q block x head pair.

                The two heads run as separate streams so each head's
                normalize chain overlaps the other head's stream. When
                proj_blocks is set (previous query block's projection), the
                h2=0 stream emits all scores+exps first, then the proj
                matmuls, then the deferred PVs — so ScalarE chews the queued
                exp backlog instead of idling while the PE projects.
                """
                pv = [psum.tile([128, 512], f32, tag="pv", bufs=2,
                                name=f"pv{h2}") for h2 in range(2)]
                for h2 in range(2):
                    hb = h2 * 64
                    defer = h2 == 0 and proj_blocks
                    pvq = []
                    for (j0, jn) in JG:          # j-groups of up to 3x128 rows
                        st = psum.tile([128, 1536], f32, tag="st", bufs=2,
                                       name="st")
                        for cx in range(jn):
                            j = j0 + cx
                            nc.tensor.matmul(
                                st[:, cx * 512:(cx + 1) * 512],
                                lhsT=kt[hp][hb:hb + 64, j * 128:(j + 1) * 128],
                                rhs=qt[hp][hb:hb + 64, qb * 512:(qb + 1) * 512],
                                start=True, stop=True,
                            )
                        et = work.tile([128, 1536], bf16, tag="et", bufs=8,
                                       name="et")
                        nc.scalar.activation(et[:, :jn * 512], st[:, :jn * 512],
                                             Exp, scale=SCALE)
                        pvq.append((et, j0, jn))
                        if not defer:
                            for (ete, pj0, pjn) in pvq:
                                for cx in range(pjn):
                                    j = pj0 + cx
                                    h = hp * 2 + h2
                                    nc.tensor.matmul(
                                        pv[h2][0:65, :],
                                        lhsT=vt[j][:, h * 65:(h + 1) * 65],
                                        rhs=ete[:, cx * 512:(cx + 1) * 512],
                                        start=(j == 0),
                                        stop=(j == N // 128 - 1),
                                    )
                            pvq = []
                    if defer:
                        for ic in proj_blocks:
                            proj_block(ic)
                        for (ete, pj0, pjn) in pvq:
                            for cx in range(pjn):
                                j = pj0 + cx
                                h = hp * 2
                                nc.tensor.matmul(
                                    pv[0][0:65, :],
                                    lhsT=vt[j][:, h * 65:(h + 1) * 65],
                                    rhs=ete[:, cx * 512:(cx + 1) * 512],
                                    start=(j == 0), stop=(j == N // 128 - 1),
                                )
                    srow = work.tile([1, 512], f32, tag="srow", bufs=4,
                                     name="srow")
                    nc.vector.tensor_copy(srow, pv[h2][64:65, :])
                    sinv = work.tile([1, 512], f32, tag="sinv", bufs=4,
                                     name="sinv")
                    nc.vector.reciprocal_approx_fast(sinv, srow)
                    bc = work.tile([64, 512], f32, tag="bc", bufs=4, name="bc")
                    nc.gpsimd.partition_broadcast(bc, sinv)
                    nc.vector.tensor_mul(
                        att[hp][h2 * 64:h2 * 64 + 64, qb * 512:(qb + 1) * 512],
                        pv[h2][0:64, :],
                        bc,
                    )

            for qb in range(N // 512):           # 512-wide query block
                for hp in range(HPC):            # head pair
                    if qb > 0 and hp < 2:
                        pblks = range((qb - 1) * 4 + 2 * hp,
                                      (qb - 1) * 4 + 2 * hp + 2)
                    else:
                        pblks = ()
                    attn_unit(qb, hp, proj_blocks=pblks)
            for ic in range(12, 16):
                proj_block(ic)

    nc.compile()
    return nc


def _get_nc():
    if "nc" not in _NC_CACHE:
        _NC_CACHE["nc"] = _build()
    return _NC_CACHE["nc"]


def _ensure_ntff_hook():
    """The agent image's ``antenv`` lacks ``axon_hooks``; synthesize it so
    ``run_bass_kernel_spmd(trace=True)`` can capture NTFF profiles."""
    import types
    try:
        from antenv.axon_hooks import get_axon_ntff_profile_hook  # noqa: F401
        return
    except ImportError:
        pass
    import antenv
    from trn_agent_boot.trn_boot import _ntff_profile_via_ctypes
    hook = _ntff_profile_via_ctypes("/opt/axon/libaxon_pjrt.so")
    mod = types.ModuleType("antenv.axon_hooks")
    mod._hook = hook
    mod.get_axon_ntff_profile_hook = lambda: mod._hook

    def _set(h):
        mod._hook = h

    mod.set_axon_ntff_profile_hook = _set
    sys.modules["antenv.axon_hooks"] = mod
    antenv.axon_hooks = mod


def kernel(trace=False, **inputs):
    x = np.asarray(inputs["x"], np.float32)
    qkv_w = np.asarray(inputs["qkv_w"], np.float32)
    proj_w = np.asarray(inputs["proj_w"], np.float32)
    proj_b = np.asarray(inputs["proj_b"], np.float32)

    nc = _get_nc()

    xTb = np.ascontiguousarray(x.transpose(0, 2, 1)).astype(ml_dtypes.bfloat16)
    wqkvT = np.ascontiguousarray(qkv_w.T).astype(ml_dtypes.bfloat16)
    wprojT = np.ascontiguousarray(proj_w.T).astype(ml_dtypes.bfloat16)

    in_maps = []
    for c in range(NCORES):
        b, hg = divmod(c, 2)
        cs = hg * CW
        wslice = np.concatenate(
            [wqkvT[:, s + cs:s + cs + CW] for s in (0, C, 2 * C)], axis=1)
        in_maps.append({
            "xT": xTb[b],
            "wqkvT": np.ascontiguousarray(wslice),
            "wprojT": np.ascontiguousarray(wprojT[cs:cs + CW, :]),
        })

    from concourse import bass_utils
    if trace:
        _ensure_ntff_hook()
        bass_utils.upload_artifacts = lambda tmpdir: tmpdir
    res = bass_utils.run_bass_kernel_spmd(
        nc, in_maps, core_ids=list(range(NCORES)), trace=trace,
    )

    out = np.empty((B, N, C), np.float32)
    for b in range(B):
        out[b] = res.results[2 * b]["out"] + res.results[2 * b + 1]["out"]
    out += proj_b

    if trace:
        return out, res
    return out
